# revision 27
# baseline (speedup 1.0000x reference)
"""Trainium2 Bass kernel for nn_FC_89094801588783.

Computes, for x[B=16, N=8192, Fin=256], W[256,256], b[256], gamma[256], beta[256]:
    y = x @ W.T + b                       (per-token Linear)
    per-sample BatchNorm over N (biased var), then gamma/beta affine.

Sharding: data-parallel over B across 8 NeuronCores (2 samples per core).

Per-core "y-once" pipeline:
  - DMA x in [128, 8, 256] tiles, token = 8p + t (8KB contiguous per
    partition), triggered from the SP HWDGE ring (sync engine).
  - PE transposes bf16 x tiles -> xT PSUM (FWL weight loads); ACT
    evacuates into a small transient ring - consumed immediately.
  - y^T = W^T-blocks (stationary bf16) @ xT (moving, N=512) -> PSUM;
    evacuated once as bf16 into a persistent y^T buffer (ACT/DVE
    split); DVE bn_stats reads the bf16 SBUF copy at 16-bit rate.
    The Linear bias b cancels in BN, so it is never loaded or added.
  - Finalize per sample: bn_aggr -> mean/var; k = gamma*rsqrt(var+eps),
    s2 = beta - mean*k. D = diag(k) as bf16; S2 = s2 broadcast to a
    natural-layout [tok, fout] tile via PE transpose.
  - Output pass: for each 128-token column block, a NORMAL matmul
    out[tok, f] = sum_p yT[p, col] * D[p, f] = y * k transposes and
    scales in one N=128 matmul. The shift S2 is prewritten into the
    (has_written-primed) PSUM banks by ACT and the start=False matmuls
    accumulate onto it. Plain-copy evacuation alternates ACT/DVE; DMA
    out (SWDGE ring on the otherwise idle GpSimd) in (p t) f layout.
  - 3 matmuls at kernel start prime the ps_o banks' has_written bits.
"""
import sys

sys.path.insert(0, "/opt/trn_rl_repo")

import numpy as np

_NC_CACHE = {}

B, N, F = 16, 8192, 256
CORES = 8
SPB = B // CORES          # samples per core = 2
TOK = SPB * N             # tokens per core = 16384
P = 128
TILES = N // 1024         # x/out DMA tiles per sample = 8
GROUPS = N // 512         # 512-col groups per sample = 16
JT = N // P               # 128-token out column blocks per sample = 64
EPS = 1e-5


def _build_nc():
    import concourse.bacc as bacc
    import concourse.tile as tile
    from concourse import mybir
    from concourse.masks import make_identity

    f32 = mybir.dt.float32
    bf16 = mybir.dt.bfloat16
    AF = mybir.ActivationFunctionType

    nc = bacc.Bacc("TRN2")
    x_d = nc.dram_tensor("x", [TOK, F], f32, kind="ExternalInput")
    w_d = nc.dram_tensor("w", [F, F], f32, kind="ExternalInput")
    g_d = nc.dram_tensor("gamma", [F], f32, kind="ExternalInput")
    be_d = nc.dram_tensor("beta", [F], f32, kind="ExternalInput")
    out_d = nc.dram_tensor("out", [TOK, F], f32, kind="ExternalOutput")

    with tile.TileContext(nc) as tc:
        with (
            tc.tile_pool(name="consts", bufs=1) as consts,
            tc.tile_pool(name="xin", bufs=4) as xin,
            tc.tile_pool(name="xtp", bufs=4) as xtp,
            tc.tile_pool(name="ytp", bufs=1) as ytp,
            tc.tile_pool(name="fin", bufs=1) as fin,
            tc.tile_pool(name="outp", bufs=3) as outp,
            tc.tile_pool(name="ps_xt", bufs=2, space="PSUM") as ps_xt,
            tc.tile_pool(name="ps_y", bufs=2, space="PSUM") as ps_y,
            tc.tile_pool(name="ps_o", bufs=3, space="PSUM") as ps_o,
        ):
            # -------- x prefetch first: DMA starts immediately --------
            # SWDGE (gpsimd) DMAs cast inline: x lands as bf16 directly.
            xpre = {}
            for i in (0, 1, 2):
                xt0 = xin.tile([P, 8, F], bf16, tag="xnat", name=f"xpre{i}")
                nc.gpsimd.dma_start(
                    out=xt0[:],
                    in_=x_d[i * 1024:(i + 1) * 1024, :].rearrange(
                        "(p t) f -> p t f", p=P),
                )
                xpre[(0, i)] = xt0

            w_sb = consts.tile([P, 2, F], f32)
            nc.sync.dma_start(out=w_sb[:], in_=w_d.rearrange("(a p) f -> p a f", p=P))
            g_col = consts.tile([P, 2], f32)
            nc.sync.dma_start(out=g_col[:], in_=g_d.rearrange("(h p) -> p h", p=P))
            be_col = consts.tile([P, 2], f32)
            nc.sync.dma_start(out=be_col[:], in_=be_d.rearrange("(h p) -> p h", p=P))

            # ---------------- constants ----------------
            ident_f = consts.tile([P, P], f32)
            make_identity(nc, ident_f)
            ident_bf = consts.tile([P, P], bf16)
            nc.vector.tensor_copy(ident_bf[:], ident_f[:])
            eps_t = consts.tile([P, 1], f32)
            nc.vector.memset(eps_t, EPS)
            zf = consts.tile([P, P], f32)
            nc.vector.memset(zf, 0.0)
            zsc = consts.tile([P, 512], bf16)
            nc.vector.memset(zsc, 0.0)

            # Prime the ps_o banks' has_written bits (full 512 cols) so the
            # ACT-path output tiles can accumulate (start=False) onto
            # ACT-prewritten shift values.
            for wu in range(3):
                pwu = ps_o.tile([P, 512], f32, tag="o", name=f"pwo{wu}")
                nc.tensor.matmul(
                    pwu[:], ident_bf[:], zsc[:], start=True, stop=True,
                )

            # W^T blocks [fin128, fout128] (c = fin chunk, a = fout half), bf16
            wT = consts.tile([P, 2, 2, P], bf16)
            for a in range(2):
                for c in range(2):
                    tp = ps_y.tile([P, P], f32, tag="w", name="wtp", bufs=1)
                    nc.tensor.transpose(tp[:], w_sb[:, a, c * P:(c + 1) * P], ident_f[:])
                    nc.scalar.copy(out=wT[:, c, a, :], in_=tp[:])

            # ---------------- per-sample state ----------------
            yt_sb = [None] * SPB          # [128, 2, 8192] bf16: y^T columns
            stats_t = [[None] * 2 for _ in range(SPB)]
            D_t = [None] * SPB            # [128, 2, 128] bf16: diag(k) halves
            S2_t = [None] * SPB           # [128, 2, 256] f32: shift, natural, x2
            for s in range(SPB):
                yt_sb[s] = ytp.tile(
                    [P, 2, N], bf16, tag=f"yt{s}", name=f"yt{s}", bufs=1
                )
                for a in range(2):
                    stats_t[s][a] = fin.tile(
                        [P, GROUPS, 6], f32, tag=f"st{s}{a}", name=f"st{s}{a}",
                        bufs=1,
                    )

            def emit_pass1(s, i):
                """1024 tokens: DMA in, transpose, y matmul, stats."""
                if (s, i) in xpre:
                    x_nat = xpre.pop((s, i))
                else:
                    x_nat = xin.tile([P, 8, F], bf16, tag="xnat")
                    tok0 = s * N + i * 1024
                    nc.gpsimd.dma_start(
                        out=x_nat[:],
                        in_=x_d[tok0:tok0 + 1024, :].rearrange(
                            "(p t) f -> p t f", p=P),
                    )
                for gl in range(2):
                    g = 2 * i + gl
                    xt = xtp.tile([P, 2, 512], bf16, tag="xt")
                    for c in range(2):
                        pxt = ps_xt.tile([P, 512], bf16, tag="xt")
                        for tt in range(4):
                            nc.tensor.matmul(
                                pxt[:, tt * P:(tt + 1) * P],
                                x_nat[:, 4 * gl + tt, c * P:(c + 1) * P],
                                ident_bf[:],
                                is_transpose=True,
                                start=(tt == 0),
                                stop=(tt == 3),
                            )
                        nc.scalar.copy(out=xt[:, c, :], in_=pxt[:])
                    for a in range(2):
                        yps = ps_y.tile([P, 512], f32, tag="y")
                        for c in range(2):
                            nc.tensor.matmul(
                                yps[:], wT[:, c, a, :], xt[:, c, :],
                                start=(c == 0), stop=(c == 1),
                            )
                        ydst = yt_sb[s][:, a, g * 512:(g + 1) * 512]
                        nc.scalar.copy(out=ydst, in_=yps[:])
                        nc.vector.bn_stats(
                            out=stats_t[s][a][:, g, :], in_=yps[:],
                        )

            def emit_finalize(s):
                """mean/var -> k, s2; build diag(k) and natural-layout S2."""
                kcol = fin.tile([P, 2], f32, tag=f"k{s}", name=f"k{s}", bufs=1)
                scol = fin.tile([P, 2], f32, tag=f"s{s}", name=f"s{s}", bufs=1)
                for a in range(2):
                    mv = fin.tile([P, 2], f32, tag=f"mv{s}", bufs=2)
                    nc.vector.bn_aggr(out=mv[:], in_=stats_t[s][a][:])
                    std = fin.tile([P, 1], f32, tag=f"std{s}", bufs=2)
                    nc.scalar.activation(
                        out=std[:], in_=mv[:, 1:2], func=AF.Sqrt,
                        bias=eps_t[:], scale=1.0,
                    )
                    nc.vector.reciprocal(out=kcol[:, a:a + 1], in_=std[:])
                    nc.vector.tensor_mul(
                        out=kcol[:, a:a + 1], in0=kcol[:, a:a + 1],
                        in1=g_col[:, a:a + 1],
                    )
                    # s2 = beta - mean*k   (Linear bias b cancels in BN)
                    sh = fin.tile([P, 1], f32, tag=f"sh{s}", bufs=2)
                    nc.vector.tensor_mul(
                        out=sh[:], in0=mv[:, 0:1], in1=kcol[:, a:a + 1]
                    )
                    nc.vector.tensor_sub(
                        out=scol[:, a:a + 1], in0=be_col[:, a:a + 1], in1=sh[:]
                    )

                D_t[s] = fin.tile([P, 2, P], bf16, tag=f"d{s}", name=f"d{s}", bufs=1)
                for a in range(2):
                    nc.vector.tensor_scalar_mul(
                        out=D_t[s][:, a, :], in0=ident_bf[:],
                        scalar1=kcol[:, a:a + 1],
                    )

                # S2 natural tile [128 tok, 256 fout]: per-partition broadcast
                # of scol along free dim, then PE transpose.
                SB = fin.tile([P, 2, P], f32, tag=f"sb{s}", name=f"sb{s}", bufs=1)
                for a in range(2):
                    nc.vector.tensor_scalar_add(
                        out=SB[:, a, :], in0=zf[:], scalar1=scol[:, a:a + 1],
                    )
                pS = ps_y.tile([P, F], f32, tag="w", name=f"psS{s}", bufs=1)
                for a in range(2):
                    nc.tensor.matmul(
                        pS[:, a * P:(a + 1) * P], SB[:, a, :], ident_f[:],
                        is_transpose=True,
                        start=(a == 0), stop=(a == 1),
                    )
                S2_t[s] = fin.tile(
                    [P, 2, F], f32, tag=f"S{s}", name=f"S{s}", bufs=1
                )
                for t2 in range(2):
                    nc.scalar.copy(out=S2_t[s][:, t2, :], in_=pS[:])

            ohold = [None]

            def emit_pass2(s, up):
                """Two 128-token column blocks u=2up,2up+1: out = y*k + s2
                via normal matmuls against diag(k), natural layout. Most
                tiles fold the +s2 into a DVE add-evacuation; every 8th
                takes the ACT path (s2 prewritten into primed PSUM, plain
                ACT copy out) to balance the two engines."""
                if up % 4 == 0:
                    ohold[0] = outp.tile([P, 8, F], f32, tag="o", name="osb")
                osb = ohold[0]
                po = ps_o.tile([P, 512], f32, tag="o")
                s2row = S2_t[s].rearrange("p t f -> p (t f)")
                act_path = (up % 32 == 16)
                if act_path:
                    nc.scalar.copy(out=po[:], in_=s2row)
                for h in range(2):
                    u = 2 * up + h
                    for a in range(2):
                        nc.tensor.matmul(
                            po[:, h * F + a * P: h * F + (a + 1) * P],
                            yt_sb[s][:, a, u * P:(u + 1) * P],
                            D_t[s][:, a, :],
                            start=(h == 0 and a == 0) and not act_path,
                            stop=(h == 1 and a == 1),
                            skip_group_check=act_path,
                        )
                t0 = (2 * up) % 8
                dst = osb[:, t0:t0 + 2, :].rearrange("p t f -> p (t f)")
                if act_path:
                    nc.scalar.copy(out=dst, in_=po[:])
                else:
                    nc.vector.tensor_add(out=dst, in0=po[:], in1=s2row)
                if up % 4 == 3:
                    row0 = s * N + (2 * up - 6) * P
                    nc.sync.dma_start(
                        out=out_d[row0:row0 + 1024, :].rearrange(
                            "(p t) f -> p t f", p=P),
                        in_=osb[:],
                    )

            # ---------------- schedule ----------------
            for i in range(TILES):
                emit_pass1(0, i)
            emit_pass1(1, 0)          # keep PE fed during finalize(0)
            emit_pass1(1, 1)
            emit_finalize(0)
            nxt = 0
            for i in range(2, TILES):
                emit_pass1(1, i)
                for up in range(nxt, nxt + 5):
                    emit_pass2(0, up)
                nxt += 5
            for up in range(nxt, JT // 2):
                emit_pass2(0, up)
            emit_finalize(1)
            for up in range(JT // 2):
                emit_pass2(1, up)

    nc.compile()
    return nc


def _get_nc():
    if "nc" not in _NC_CACHE:
        _NC_CACHE["nc"] = _build_nc()
    return _NC_CACHE["nc"]


def make_in_maps(x, W, gamma, beta):
    shards = np.asarray(x, dtype=np.float32).reshape(CORES, TOK, F)
    W = np.asarray(W, dtype=np.float32)
    gamma = np.asarray(gamma, dtype=np.float32)
    beta = np.asarray(beta, dtype=np.float32)
    return [
        {
            "x": np.ascontiguousarray(shards[i]),
            "w": W, "gamma": gamma, "beta": beta,
        }
        for i in range(CORES)
    ]


def kernel(x, W, b, gamma, beta):
    from concourse.bass_utils import run_bass_kernel_spmd

    nc = _get_nc()
    in_maps = make_in_maps(x, W, gamma, beta)
    try:
        res = run_bass_kernel_spmd(nc, in_maps, core_ids=list(range(CORES)))
    except Exception:
        # One retry: a previous crashed run can leave a core wedged.
        res = run_bass_kernel_spmd(nc, in_maps, core_ids=list(range(CORES)))
    out = np.stack([res.results[i]["out"] for i in range(CORES)])
    return out.reshape(B, N, F).astype(np.float32)


if __name__ == "__main__":
    rng = np.random.default_rng(0)
    x = rng.standard_normal((B, N, F), dtype=np.float32)
    W = ((rng.random((F, F), dtype=np.float32) - 0.5) / 8).astype(np.float32)
    b = ((rng.random(F, dtype=np.float32) - 0.5) / 8).astype(np.float32)
    gamma = np.ones(F, np.float32)
    beta = np.zeros(F, np.float32)
    out = kernel(x=x, W=W, b=b, gamma=gamma, beta=beta)
    y = x @ W.T + b
    mean = y.mean(axis=1, keepdims=True)
    var = ((y - mean) ** 2).mean(axis=1, keepdims=True)
    ref = (y - mean) / np.sqrt(var + EPS) * gamma + beta
    err = np.abs(out - ref).max()
    print("maxabs err:", err, "rel:", err / np.abs(ref).max())


# revision 29
# speedup vs baseline: 1.1647x; 1.1647x over previous
"""Trainium2 Bass kernel for nn_FC_89094801588783.

Computes, for x[B=16, N=8192, Fin=256], W[256,256], b[256], gamma[256], beta[256]:
    y = x @ W.T + b                       (per-token Linear)
    per-sample BatchNorm over N (biased var), then gamma/beta affine.

Sharding: data-parallel over B across 8 NeuronCores (2 samples per core).

Per-core "y-once" pipeline:
  - DMA x in [128, 8, 256] tiles, token = 8p + t (8KB contiguous per
    partition), triggered from the SP HWDGE ring (sync engine).
  - PE transposes bf16 x tiles -> xT PSUM (FWL weight loads); ACT
    evacuates into a small transient ring - consumed immediately.
  - y^T = W^T-blocks (stationary bf16) @ xT (moving, N=512) -> PSUM;
    evacuated once as bf16 into a persistent y^T buffer (ACT/DVE
    split); DVE bn_stats reads the bf16 SBUF copy at 16-bit rate.
    The Linear bias b cancels in BN, so it is never loaded or added.
  - Finalize per sample: bn_aggr -> mean/var; k = gamma*rsqrt(var+eps),
    s2 = beta - mean*k. D = diag(k) as bf16; S2 = s2 broadcast to a
    natural-layout [tok, fout] tile via PE transpose.
  - Output pass: for each 128-token column block, a NORMAL matmul
    out[tok, f] = sum_p yT[p, col] * D[p, f] = y * k transposes and
    scales in one N=128 matmul. The shift S2 is prewritten into the
    (has_written-primed) PSUM banks by ACT and the start=False matmuls
    accumulate onto it. Plain-copy evacuation alternates ACT/DVE; DMA
    out (SWDGE ring on the otherwise idle GpSimd) in (p t) f layout.
  - 3 matmuls at kernel start prime the ps_o banks' has_written bits.
"""
import sys

sys.path.insert(0, "/opt/trn_rl_repo")

import numpy as np

_NC_CACHE = {}

B, N, F = 16, 8192, 256
CORES = 8
SPB = B // CORES          # samples per core = 2
TOK = SPB * N             # tokens per core = 16384
P = 128
TILES = N // 1024         # x/out DMA tiles per sample = 8
GROUPS = N // 512         # 512-col groups per sample = 16
JT = N // P               # 128-token out column blocks per sample = 64
EPS = 1e-5


def _build_nc():
    import concourse.bacc as bacc
    import concourse.tile as tile
    from concourse import mybir
    from concourse.masks import make_identity

    f32 = mybir.dt.float32
    bf16 = mybir.dt.bfloat16
    AF = mybir.ActivationFunctionType

    nc = bacc.Bacc("TRN2")
    x_d = nc.dram_tensor("x", [TOK, F], f32, kind="ExternalInput")
    w_d = nc.dram_tensor("w", [F, F], f32, kind="ExternalInput")
    g_d = nc.dram_tensor("gamma", [F], f32, kind="ExternalInput")
    be_d = nc.dram_tensor("beta", [F], f32, kind="ExternalInput")
    out_d = nc.dram_tensor("out", [TOK, F], f32, kind="ExternalOutput")

    with tile.TileContext(nc) as tc:
        with (
            tc.tile_pool(name="consts", bufs=1) as consts,
            tc.tile_pool(name="xin", bufs=4) as xin,
            tc.tile_pool(name="xtp", bufs=4) as xtp,
            tc.tile_pool(name="ytp", bufs=1) as ytp,
            tc.tile_pool(name="fin", bufs=1) as fin,
            tc.tile_pool(name="outp", bufs=3) as outp,
            tc.tile_pool(name="ps_xt", bufs=2, space="PSUM") as ps_xt,
            tc.tile_pool(name="ps_y", bufs=2, space="PSUM") as ps_y,
            tc.tile_pool(name="ps_o", bufs=3, space="PSUM") as ps_o,
        ):
            # -------- x prefetch first: DMA starts immediately --------
            # SWDGE (gpsimd) DMAs cast inline: x lands as bf16 directly.
            xpre = {}
            for i in (0, 1, 2):
                xt0 = xin.tile([P, 8, F], bf16, tag="xnat", name=f"xpre{i}")
                nc.gpsimd.dma_start(
                    out=xt0[:],
                    in_=x_d[i * 1024:(i + 1) * 1024, :].rearrange(
                        "(p t) f -> p t f", p=P),
                )
                xpre[(0, i)] = xt0

            w_sb = consts.tile([P, 2, F], f32)
            nc.sync.dma_start(out=w_sb[:], in_=w_d.rearrange("(a p) f -> p a f", p=P))
            g_col = consts.tile([P, 2], f32)
            nc.sync.dma_start(out=g_col[:], in_=g_d.rearrange("(h p) -> p h", p=P))
            be_col = consts.tile([P, 2], f32)
            nc.sync.dma_start(out=be_col[:], in_=be_d.rearrange("(h p) -> p h", p=P))

            # ---------------- constants ----------------
            ident_f = consts.tile([P, P], f32)
            make_identity(nc, ident_f)
            ident_bf = consts.tile([P, P], bf16)
            nc.vector.tensor_copy(ident_bf[:], ident_f[:])
            eps_t = consts.tile([P, 1], f32)
            nc.vector.memset(eps_t, EPS)
            zf = consts.tile([P, P], f32)
            nc.vector.memset(zf, 0.0)
            zsc = consts.tile([P, 512], bf16)
            nc.vector.memset(zsc, 0.0)

            # Prime the ps_o banks' has_written bits (full 512 cols) so the
            # ACT-path output tiles can accumulate (start=False) onto
            # ACT-prewritten shift values.
            for wu in range(3):
                pwu = ps_o.tile([P, 512], f32, tag="o", name=f"pwo{wu}")
                nc.tensor.matmul(
                    pwu[:], ident_bf[:], zsc[:], start=True, stop=True,
                )

            # W^T blocks [fin128, fout128] (c = fin chunk, a = fout half), bf16
            wT = consts.tile([P, 2, 2, P], bf16)
            for a in range(2):
                for c in range(2):
                    tp = ps_y.tile([P, P], f32, tag="w", name="wtp", bufs=1)
                    nc.tensor.transpose(tp[:], w_sb[:, a, c * P:(c + 1) * P], ident_f[:])
                    nc.scalar.copy(out=wT[:, c, a, :], in_=tp[:])

            # ---------------- per-sample state ----------------
            yt_sb = [None] * SPB          # [128, 2, 8192] bf16: y^T columns
            stats_t = [[None] * 2 for _ in range(SPB)]
            D_t = [None] * SPB            # [128, 2, 128] bf16: diag(k) halves
            S2_t = [None] * SPB           # [128, 2, 256] f32: shift, natural, x2
            for s in range(SPB):
                yt_sb[s] = ytp.tile(
                    [P, 2, N], bf16, tag=f"yt{s}", name=f"yt{s}", bufs=1
                )
                for a in range(2):
                    stats_t[s][a] = fin.tile(
                        [P, GROUPS, 6], f32, tag=f"st{s}{a}", name=f"st{s}{a}",
                        bufs=1,
                    )

            def emit_pass1(s, i):
                """1024 tokens: DMA in, transpose, y matmul, stats."""
                if (s, i) in xpre:
                    x_nat = xpre.pop((s, i))
                else:
                    x_nat = xin.tile([P, 8, F], bf16, tag="xnat")
                    tok0 = s * N + i * 1024
                    nc.gpsimd.dma_start(
                        out=x_nat[:],
                        in_=x_d[tok0:tok0 + 1024, :].rearrange(
                            "(p t) f -> p t f", p=P),
                    )
                for gl in range(2):
                    g = 2 * i + gl
                    xt = xtp.tile([P, 2, 512], bf16, tag="xt")
                    for c in range(2):
                        pxt = ps_xt.tile([P, 512], bf16, tag="xt")
                        for tt in range(4):
                            nc.tensor.matmul(
                                pxt[:, tt * P:(tt + 1) * P],
                                x_nat[:, 4 * gl + tt, c * P:(c + 1) * P],
                                ident_bf[:],
                                is_transpose=True,
                                start=(tt == 0),
                                stop=(tt == 3),
                            )
                        nc.scalar.copy(out=xt[:, c, :], in_=pxt[:])
                    for a in range(2):
                        yps = ps_y.tile([P, 512], f32, tag="y")
                        for c in range(2):
                            nc.tensor.matmul(
                                yps[:], wT[:, c, a, :], xt[:, c, :],
                                start=(c == 0), stop=(c == 1),
                            )
                        ydst = yt_sb[s][:, a, g * 512:(g + 1) * 512]
                        nc.scalar.copy(out=ydst, in_=yps[:])
                        nc.vector.bn_stats(
                            out=stats_t[s][a][:, g, :], in_=ydst,
                        )

            def emit_finalize(s):
                """mean/var -> k, s2; build diag(k) and natural-layout S2."""
                kcol = fin.tile([P, 2], f32, tag=f"k{s}", name=f"k{s}", bufs=1)
                scol = fin.tile([P, 2], f32, tag=f"s{s}", name=f"s{s}", bufs=1)
                for a in range(2):
                    mv = fin.tile([P, 2], f32, tag=f"mv{s}", bufs=2)
                    nc.vector.bn_aggr(out=mv[:], in_=stats_t[s][a][:])
                    std = fin.tile([P, 1], f32, tag=f"std{s}", bufs=2)
                    nc.scalar.activation(
                        out=std[:], in_=mv[:, 1:2], func=AF.Sqrt,
                        bias=eps_t[:], scale=1.0,
                    )
                    nc.vector.reciprocal(out=kcol[:, a:a + 1], in_=std[:])
                    nc.vector.tensor_mul(
                        out=kcol[:, a:a + 1], in0=kcol[:, a:a + 1],
                        in1=g_col[:, a:a + 1],
                    )
                    # s2 = beta - mean*k   (Linear bias b cancels in BN)
                    sh = fin.tile([P, 1], f32, tag=f"sh{s}", bufs=2)
                    nc.vector.tensor_mul(
                        out=sh[:], in0=mv[:, 0:1], in1=kcol[:, a:a + 1]
                    )
                    nc.vector.tensor_sub(
                        out=scol[:, a:a + 1], in0=be_col[:, a:a + 1], in1=sh[:]
                    )

                D_t[s] = fin.tile([P, 2, P], bf16, tag=f"d{s}", name=f"d{s}", bufs=1)
                for a in range(2):
                    nc.vector.tensor_scalar_mul(
                        out=D_t[s][:, a, :], in0=ident_bf[:],
                        scalar1=kcol[:, a:a + 1],
                    )

                # S2 natural tile [128 tok, 256 fout]: per-partition broadcast
                # of scol along free dim, then PE transpose.
                SB = fin.tile([P, 2, P], f32, tag=f"sb{s}", name=f"sb{s}", bufs=1)
                for a in range(2):
                    nc.vector.tensor_scalar_add(
                        out=SB[:, a, :], in0=zf[:], scalar1=scol[:, a:a + 1],
                    )
                pS = ps_y.tile([P, F], f32, tag="w", name=f"psS{s}", bufs=1)
                for a in range(2):
                    nc.tensor.matmul(
                        pS[:, a * P:(a + 1) * P], SB[:, a, :], ident_f[:],
                        is_transpose=True,
                        start=(a == 0), stop=(a == 1),
                    )
                S2_t[s] = fin.tile(
                    [P, 2, F], f32, tag=f"S{s}", name=f"S{s}", bufs=1
                )
                for t2 in range(2):
                    nc.scalar.copy(out=S2_t[s][:, t2, :], in_=pS[:])

            ohold = [None]

            def emit_pass2(s, up):
                """Two 128-token column blocks u=2up,2up+1: out = y*k + s2
                via normal matmuls against diag(k), natural layout. Most
                tiles fold the +s2 into a DVE add-evacuation; every 8th
                takes the ACT path (s2 prewritten into primed PSUM, plain
                ACT copy out) to balance the two engines."""
                if up % 4 == 0:
                    ohold[0] = outp.tile([P, 8, F], f32, tag="o", name="osb")
                osb = ohold[0]
                po = ps_o.tile([P, 512], f32, tag="o")
                s2row = S2_t[s].rearrange("p t f -> p (t f)")
                act_path = (up % 32 == 16)
                if act_path:
                    nc.scalar.copy(out=po[:], in_=s2row)
                for h in range(2):
                    u = 2 * up + h
                    for a in range(2):
                        nc.tensor.matmul(
                            po[:, h * F + a * P: h * F + (a + 1) * P],
                            yt_sb[s][:, a, u * P:(u + 1) * P],
                            D_t[s][:, a, :],
                            start=(h == 0 and a == 0) and not act_path,
                            stop=(h == 1 and a == 1),
                            skip_group_check=act_path,
                        )
                t0 = (2 * up) % 8
                dst = osb[:, t0:t0 + 2, :].rearrange("p t f -> p (t f)")
                if act_path:
                    nc.scalar.copy(out=dst, in_=po[:])
                else:
                    nc.vector.tensor_add(out=dst, in0=po[:], in1=s2row)
                if up % 4 == 3:
                    row0 = s * N + (2 * up - 6) * P
                    nc.sync.dma_start(
                        out=out_d[row0:row0 + 1024, :].rearrange(
                            "(p t) f -> p t f", p=P),
                        in_=osb[:],
                    )

            # ---------------- schedule ----------------
            for i in range(TILES):
                emit_pass1(0, i)
            emit_pass1(1, 0)          # keep PE fed during finalize(0)
            emit_finalize(0)
            for i in range(1, TILES):
                emit_pass1(1, i)
                for up in range(4 * (i - 1), 4 * i):
                    emit_pass2(0, up)
            for up in range(4 * (TILES - 1), JT // 2):
                emit_pass2(0, up)
            emit_finalize(1)
            for up in range(JT // 2):
                emit_pass2(1, up)

    nc.compile()
    return nc


def _get_nc():
    if "nc" not in _NC_CACHE:
        _NC_CACHE["nc"] = _build_nc()
    return _NC_CACHE["nc"]


def make_in_maps(x, W, gamma, beta):
    shards = np.asarray(x, dtype=np.float32).reshape(CORES, TOK, F)
    W = np.asarray(W, dtype=np.float32)
    gamma = np.asarray(gamma, dtype=np.float32)
    beta = np.asarray(beta, dtype=np.float32)
    return [
        {
            "x": np.ascontiguousarray(shards[i]),
            "w": W, "gamma": gamma, "beta": beta,
        }
        for i in range(CORES)
    ]


def kernel(x, W, b, gamma, beta):
    from concourse.bass_utils import run_bass_kernel_spmd

    nc = _get_nc()
    in_maps = make_in_maps(x, W, gamma, beta)
    try:
        res = run_bass_kernel_spmd(nc, in_maps, core_ids=list(range(CORES)))
    except Exception:
        # One retry: a previous crashed run can leave a core wedged.
        res = run_bass_kernel_spmd(nc, in_maps, core_ids=list(range(CORES)))
    out = np.stack([res.results[i]["out"] for i in range(CORES)])
    return out.reshape(B, N, F).astype(np.float32)


if __name__ == "__main__":
    rng = np.random.default_rng(0)
    x = rng.standard_normal((B, N, F), dtype=np.float32)
    W = ((rng.random((F, F), dtype=np.float32) - 0.5) / 8).astype(np.float32)
    b = ((rng.random(F, dtype=np.float32) - 0.5) / 8).astype(np.float32)
    gamma = np.ones(F, np.float32)
    beta = np.zeros(F, np.float32)
    out = kernel(x=x, W=W, b=b, gamma=gamma, beta=beta)
    y = x @ W.T + b
    mean = y.mean(axis=1, keepdims=True)
    var = ((y - mean) ** 2).mean(axis=1, keepdims=True)
    ref = (y - mean) / np.sqrt(var + EPS) * gamma + beta
    err = np.abs(out - ref).max()
    print("maxabs err:", err, "rel:", err / np.abs(ref).max())
